# revision 16
# baseline (speedup 1.0000x reference)
"""Causal self-attention (B=4, T=2048, C=1024, H=16, D=64) on 8 TRN2 NeuronCores.

Sharding: tensor-parallel over heads. Each core owns a 128-channel slice of the
QKV projection outputs (= 2 heads) and the matching 128-row slice of the output
projection. Each core produces a full-shape partial output y_c = attn_c @ Wp_c;
the host sums the 8 partials and adds bp.

Per-core program (SPMD, one NEFF):
  phase 1: Q^T/K^T/V^T = W_s^T @ x^T   (tiles of 512 tokens, K=1024 contraction)
  phase 2: per (batch, tq-block of 512):
     S^T tiles [128 tk, 512 tq] per head (2 heads row-packed in the PE array),
     causal mask on diagonal 128-blocks, exp on ACT (scale=1/8 folded in),
     O~^T accumulated over tk tiles with lhsT = [V_tile | ones] so row 64 of
     the accumulator is the softmax denominator; normalize via PE broadcast of
     the reciprocal row; per-batch output projection and DMA out.

All matmuls run as float32r (1 cycle/row when N>=256 on TRN2).
"""

import functools
import os
import sys

sys.path.insert(0, "/opt/trn_rl_repo")

import numpy as np

B, T, C = 4, 2048, 1024
H, D = 16, 64
NCORES = 8
CS = C // NCORES          # 128 channels per core = 2 heads
HL = CS // D              # 2 local heads
NTOK = B * T              # 8192
NKT = C // 128            # 8 contraction tiles for the projections
TB = 512                  # token block (phase 1) and tq block (phase 2)
NTB = NTOK // TB          # 16
NQB = T // TB             # 4 tq blocks per batch
NKTILE = T // 128         # 16 tk tiles per batch
SCALE = 1.0 / np.sqrt(D)  # 0.125
NEG = -1.0e9


@functools.lru_cache(maxsize=1)
def _build():
    import concourse.bass as bass
    import concourse.tile as tile
    from concourse import bacc, mybir

    dt = mybir.dt
    F32 = dt.float32
    F32R = dt.float32r
    AF = mybir.ActivationFunctionType
    OP = mybir.AluOpType

    nc = bacc.Bacc(
        "TRN2",
        target_bir_lowering=False,
        debug=False,
        enable_asserts=False,
        num_devices=NCORES,
    )

    xt = nc.dram_tensor("xt", (C, NTOK), F32R, kind="ExternalInput").ap()
    wq_d = nc.dram_tensor("wq", (128, NKT * 128), F32R, kind="ExternalInput").ap()
    wk_d = nc.dram_tensor("wk", (128, NKT * 128), F32R, kind="ExternalInput").ap()
    wv_d = nc.dram_tensor("wv", (128, NKT * 128), F32R, kind="ExternalInput").ap()
    wp_d = nc.dram_tensor("wp", (CS, C), F32R, kind="ExternalInput").ap()
    bq_d = nc.dram_tensor("bq", (CS, 1), F32, kind="ExternalInput").ap()
    bk_d = nc.dram_tensor("bk", (CS, 1), F32, kind="ExternalInput").ap()
    bv_d = nc.dram_tensor("bv", (CS, 1), F32, kind="ExternalInput").ap()
    ident_d = nc.dram_tensor("ident", (128, 128), F32R, kind="ExternalInput").ap()
    trim_d = nc.dram_tensor("trim", (128, 128), F32, kind="ExternalInput").ap()
    ones_d = nc.dram_tensor("ones", (1, 64), F32R, kind="ExternalInput").ap()
    y_d = nc.dram_tensor("y", (NTOK, C), F32, kind="ExternalOutput").ap()

    with tile.TileContext(nc) as tc:
        import contextlib

        with contextlib.ExitStack() as ctx:
            persist = ctx.enter_context(tc.tile_pool(name="persist", bufs=1))
            psbig = ctx.enter_context(tc.tile_pool(name="psbig", bufs=2, space="PSUM"))
            pso = ctx.enter_context(tc.tile_pool(name="pso", bufs=3, space="PSUM"))
            psbc = ctx.enter_context(tc.tile_pool(name="psbc", bufs=1, space="PSUM"))

            qt = persist.tile([128, NTOK], F32R, tag="qt")
            kt = persist.tile([128, NTOK], F32R, tag="kt")
            vt = persist.tile([128, NTOK], F32R, tag="vt")
            at = persist.tile([128, NTOK], F32R, tag="at")

            wp = persist.tile([CS, C], F32R, tag="wp")
            bq = persist.tile([CS, 1], F32, tag="bq")
            bk = persist.tile([CS, 1], F32, tag="bk")
            bv = persist.tile([CS, 1], F32, tag="bv")
            ident = persist.tile([128, 128], F32R, tag="ident")
            trim = persist.tile([128, 128], F32, tag="trim")
            ones64 = persist.tile([1, 64], F32R, tag="ones64")
            onescol = persist.tile([128, 1], F32, tag="onescol")
            zeros = persist.tile([128, 384], F32, tag="zeros")
            nc.vector.memset(onescol[:], 1.0)
            nc.vector.memset(zeros[:], 0.0)

            nc.sync.dma_start(wp[:], wp_d[:])
            nc.sync.dma_start(bq[:], bq_d[:])
            nc.sync.dma_start(bk[:], bk_d[:])
            nc.sync.dma_start(bv[:], bv_d[:])

            nc.sync.dma_start(ident[:], ident_d[:])
            nc.sync.dma_start(trim[:], trim_d[:])
            nc.sync.dma_start(ones64[:], ones_d[:])

            # ---------------- phase 1: projections ----------------
            with tc.tile_pool(name="xbp", bufs=22) as xbp:
                wq = xbp.tile([128, NKT * 128], F32R, tag="wq", bufs=1)
                wk = xbp.tile([128, NKT * 128], F32R, tag="wk", bufs=1)
                wv = xbp.tile([128, NKT * 128], F32R, tag="wv", bufs=1)
                nc.sync.dma_start(wq[:], wq_d[:])
                nc.sync.dma_start(wk[:], wk_d[:])
                nc.sync.dma_start(wv[:], wv_d[:])
                for tb in range(NTB):
                    xbt = []
                    for k in range(NKT):
                        xb = xbp.tile([128, TB], F32R, tag="xb")
                        nc.sync.dma_start(
                            xb[:],
                            xt[k * 128:(k + 1) * 128, tb * TB:(tb + 1) * TB],
                        )
                        xbt.append(xb)
                    for (w_sb, b_sb, dst) in ((wq, bq, qt), (wk, bk, kt), (wv, bv, vt)):
                        ps = pso.tile([128, TB], F32, tag="o", name="ps_proj")
                        for k in range(NKT):
                            nc.tensor.matmul(
                                ps[:],
                                w_sb[:, k * 128:(k + 1) * 128],
                                xbt[k][:],
                                start=(k == 0),
                                stop=(k == NKT - 1),
                            )
                        nc.vector.tensor_scalar_add(
                            dst[:, tb * TB:(tb + 1) * TB], ps[:], b_sb[:, 0:1]
                        )

            # ---------------- phase 2: attention + output projection ----------
            with contextlib.ExitStack() as ctx2:
                vxp = ctx2.enter_context(tc.tile_pool(name="vxp", bufs=3))
                ppl = ctx2.enter_context(tc.tile_pool(name="ppl", bufs=4))
                ysp = ctx2.enter_context(tc.tile_pool(name="ysp", bufs=3))
                bcs = ctx2.enter_context(tc.tile_pool(name="bcs", bufs=3))
                rcp = ctx2.enter_context(tc.tile_pool(name="rcp", bufs=4))

                for b in range(B):
                    tok0 = b * T
                    # Build Vx = [V_tile | ones] per head: [128 tk, 65] x 16 tiles
                    vx = []
                    for h in range(HL):
                        vxh = vxp.tile([128, NKTILE * 65], F32R, tag="vx", name=f"vx{h}")
                        vx.append(vxh)
                        vview = vxh.rearrange("p (i c) -> p i c", c=65)
                        nc.vector.tensor_copy(
                            vview[:, :, 64:65],
                            onescol[:].unsqueeze(1).broadcast_to([128, NKTILE, 1]),
                        )
                        hs = slice(h * D, (h + 1) * D)
                        for g in range(2):
                            tp = psbc.tile([128, 512], F32R, tag="bc", name="tp")
                            for t in range(8):
                                i = g * 8 + t
                                nc.tensor.transpose(
                                    tp[:, t * 64:(t + 1) * 64],
                                    vt[hs, tok0 + i * 128: tok0 + (i + 1) * 128],
                                    ident[hs, hs],
                                )
                            nc.vector.tensor_copy(
                                vview[:, g * 8:(g + 1) * 8, 0:64],
                                tp[:, 0:512].rearrange("p (i c) -> p i c", c=64),
                            )

                    for j in range(NQB):
                        ntk = 4 * j + 4
                        tqs = slice(tok0 + j * TB, tok0 + (j + 1) * TB)
                        ov = []
                        for h in range(HL):
                            oh = pso.tile([128, TB], F32, tag="o", name=f"o{h}")
                            ov.append(oh)
                        for i in range(ntk):
                            tks = slice(tok0 + i * 128, tok0 + (i + 1) * 128)
                            sp = psbig.tile([128, 1024], F32, tag="s", name="sp")
                            for h in range(HL):
                                hs = slice(h * D, (h + 1) * D)
                                nc.tensor.matmul(
                                    sp[:, h * TB:(h + 1) * TB],
                                    kt[hs, tks],
                                    qt[hs, tqs],
                                    start=True,
                                    stop=True,
                                )
                            pp = ppl.tile([128, 1024], F32R, tag="p", name="pp")
                            r = i - 4 * j
                            if r < 0:
                                nc.scalar.activation(pp[:], sp[:], AF.Exp, scale=SCALE)
                            else:
                                spv = sp.rearrange("p (h c) -> p h c", c=TB)
                                ppv = pp.rearrange("p (h c) -> p h c", c=TB)
                                if r > 0:
                                    nc.vector.tensor_copy(
                                        ppv[:, :, 0:128 * r],
                                        zeros[:, 0:128 * r]
                                        .unsqueeze(1)
                                        .broadcast_to([128, HL, 128 * r]),
                                    )
                                trb = trim[:].unsqueeze(1).broadcast_to([128, HL, 128])
                                nc.vector.tensor_tensor(
                                    spv[:, :, 128 * r:128 * (r + 1)],
                                    spv[:, :, 128 * r:128 * (r + 1)],
                                    trb,
                                    op=OP.add,
                                )
                                nc.scalar.activation(
                                    ppv[:, :, 128 * r:TB],
                                    spv[:, :, 128 * r:TB],
                                    AF.Exp,
                                    scale=SCALE,
                                )
                            for h in range(HL):
                                nc.tensor.matmul(
                                    ov[h][0:65, :],
                                    vx[h][:, i * 65:(i + 1) * 65],
                                    pp[:, h * TB:(h + 1) * TB],
                                    start=(i == 0),
                                    stop=(i == ntk - 1),
                                )
                        for h in range(HL):
                            rec = rcp.tile([1, TB], F32R, tag="rec", name="rec")
                            with nc.allow_low_precision(reason="f32r recip"):
                                nc.vector.reciprocal(rec[:], ov[h][64:65, :])
                            bcp = psbc.tile([64, TB], F32, tag="bc", name="bcp")
                            nc.tensor.matmul(
                                bcp[:],
                                ones64[:],
                                rec[:],
                                start=True,
                                stop=True,
                            )
                            bcst = bcs.tile([64, TB], F32, tag="bcs", name="bcst")
                            nc.vector.tensor_copy(bcst[:], bcp[:])
                            nc.vector.tensor_tensor(
                                at[h * D:(h + 1) * D, tqs],
                                ov[h][0:64, :],
                                bcst[:],
                                op=OP.mult,
                            )

                    # ---- output projection for batch b ----
                    for tt in range(T // 128):
                        rows = slice(tok0 + tt * 128, tok0 + (tt + 1) * 128)
                        yp = psbig.tile([128, 1024], F32, tag="s", name="yp")
                        for n in range(2):
                            nc.tensor.matmul(
                                yp[:, n * TB:(n + 1) * TB],
                                at[:, rows],
                                wp[:, n * TB:(n + 1) * TB],
                                start=True,
                                stop=True,
                            )
                        ys = ysp.tile([128, 1024], F32, tag="ys", name="ys")
                        nc.vector.tensor_copy(ys[:], yp[:])
                        nc.sync.dma_start(y_d[rows, :], ys[:])

    nc.compile()
    return nc


def _prep_inputs(inputs):
    """Host-side sharding: returns (in_maps list of 8 dicts, bp)."""
    x = np.asarray(inputs["x"], dtype=np.float32)
    xt = np.ascontiguousarray(x.reshape(NTOK, C).T)

    def pretile(w):  # (C, 128) col-slice -> [128, NKT*128] k-major tiles
        return np.ascontiguousarray(
            w.reshape(NKT, 128, 128).transpose(1, 0, 2).reshape(128, NKT * 128)
        )

    ident = np.eye(128, dtype=np.float32)
    # S^T-layout causal mask for diagonal blocks: rows = tk, cols = tq.
    # visible (keep 0) iff tq >= tk, else NEG.
    trim = np.where(
        np.arange(128)[None, :] >= np.arange(128)[:, None], 0.0, NEG
    ).astype(np.float32)
    ones = np.ones((1, 64), dtype=np.float32)

    in_maps = []
    for c in range(NCORES):
        cs = slice(c * CS, (c + 1) * CS)
        m = {
            "xt": xt,
            "ident": ident,
            "trim": trim,
            "ones": ones,
            "wq": pretile(np.asarray(inputs["Wq"], np.float32)[:, cs]),
            "wk": pretile(np.asarray(inputs["Wk"], np.float32)[:, cs]),
            "wv": pretile(np.asarray(inputs["Wv"], np.float32)[:, cs]),
            "wp": np.ascontiguousarray(np.asarray(inputs["Wp"], np.float32)[cs, :]),
            "bq": np.ascontiguousarray(np.asarray(inputs["bq"], np.float32)[cs, None]),
            "bk": np.ascontiguousarray(np.asarray(inputs["bk"], np.float32)[cs, None]),
            "bv": np.ascontiguousarray(np.asarray(inputs["bv"], np.float32)[cs, None]),
        }
        in_maps.append(m)
    return in_maps, np.asarray(inputs["bp"], np.float32)


def _run(inputs, **kw):
    from concourse import bass_utils

    nc = _build()
    in_maps, bp = _prep_inputs(inputs)
    res = bass_utils.run_bass_kernel_spmd(
        nc, in_maps, core_ids=list(range(NCORES)), **kw
    )
    acc = np.zeros((NTOK, C), dtype=np.float32)
    for r in res.results:
        acc += r["y"]
    acc += bp[None, :]
    return acc.reshape(B, T, C), res


def kernel(**inputs):
    out, _ = _run(inputs)
    return out


if __name__ == "__main__":
    # smoke test with tiny random data through the simulator is not supported
    # here; use test.py on hardware.
    nc = _build()
    print("built ok:", nc)
